# revision 52
# baseline (speedup 1.0000x reference)
"""Trainium2 Bass kernel for LlamaAttention (B=1, S=2048, HID=2048, H=32, KV=8, D=64).

Sharding (8 cores): tensor-parallel over heads. Core c owns q-heads 4c..4c+3 and
kv-head c. Each core computes QKV projections for its heads, RoPE, causal
attention; attention outputs are AllGathered per 512-column chunk (bf16) while
later chunks are still being computed, and each core computes its 256 output
features of o_proj (Wo column shard). Host concatenates the 8 shards.

Layout/precision choices:
  - Weights + hidden activations are bf16 (matmuls accumulate fp32 in PSUM).
  - RoPE pairs are interleaved on partitions (dim i -> 2i, dim i+32 -> 2i+1) so
    rotate_half becomes an even/odd partition swap done by DVE stream_shuffle.
    cos/sin tables (sign baked into sin) are computed on host from
    position_ids/powers.
  - Attention is software-pipelined: scores MMs stream over ki with av MMs
    lagging LAG ki behind. Score PSUM tiles drain via DVE (hp0, staged f32 then
    ACT exp) and via direct ACT exp (hp1), so the PE never blocks on one
    engine. Causal masking is fused into the drain (hp0) or applied in-place
    (hp1). Softmax normalization is deferred: av accumulates unnormalized rows
    plus a ones-row giving Z; ACT computes 1/Z from PSUM directly.
"""

import numpy as np
import ml_dtypes

import concourse.bass as bass
import concourse.mybir as mybir
import concourse.tile as tile
from concourse import bacc
from concourse import bass_utils
from concourse.bass_interp import get_hw_module

S = 2048
HID = 2048
H = 32
KV = 8
D = 64
NCORES = 8
HQ = H // NCORES          # 4 q heads per core
BASE = 10000.0
F32 = mybir.dt.float32
BF16 = mybir.dt.bfloat16
AF = mybir.ActivationFunctionType
ST = S // 512             # 4 s/q tiles of 512
KO = HID // 128           # 16 contraction chunks
NEG = -1.0e30
LAG = 4                   # av pipeline lag (in ki chunks) behind scores
SWAP_MASK = [i ^ 1 for i in range(32)]  # even/odd partition swap per quadrant


def build_body(tc, aps):
    nc = tc.nc
    hT3 = aps["hiddenT"].rearrange("(ko p) s -> p ko s", p=128)
    wq3 = aps["wqkvT"].rearrange("(ko p) m -> p ko m", p=128)
    wo3 = aps["woT"].rearrange("(ko p) m -> p ko m", p=128)
    outT = aps["outT"]

    from contextlib import ExitStack
    es = ExitStack()
    const_pool = es.enter_context(tc.tile_pool(name="const", bufs=1))
    persist = es.enter_context(tc.tile_pool(name="persist", bufs=1))
    dram = es.enter_context(tc.tile_pool(name="dram", bufs=1, space="DRAM"))

    # ---- constants (spread across the SP/ACT/Pool DMA queues so the
    # startup loads stream in parallel: SP carries ht, ACT carries wq/wo,
    # Pool carries mask/ident/tables) ----
    mask_sb = const_pool.tile([128, 128], F32, tag="mask")
    nc.gpsimd.dma_start(mask_sb[:], aps["trimask"][:])
    identhi = const_pool.tile([128, 64], BF16, tag="identhi")
    nc.gpsimd.dma_start(identhi[:], aps["identhi"][:])
    cosI = const_pool.tile([128, S], F32, tag="cos")
    nc.gpsimd.dma_start(cosI[:], aps["cos_t"][:])
    sinI = const_pool.tile([128, S], F32, tag="sin")
    nc.gpsimd.dma_start(sinI[:], aps["sin_t"][:])

    # ---- persistent activations ----
    # kv weight columns first (first chain to run), then q columns
    wq_sb = persist.tile([128, KO, 384], BF16, tag="wq")
    for k0 in range(0, KO, 4):
        nc.scalar.dma_start(wq_sb[:, k0:k0 + 4, 256:384], wq3[:, k0:k0 + 4, 256:384])
    for k0 in range(0, KO, 8):
        nc.scalar.dma_start(wq_sb[:, k0:k0 + 8, 0:256], wq3[:, k0:k0 + 8, 0:256])
    wo_sb = persist.tile([128, KO, 256], BF16, tag="wo")
    nc.scalar.dma_start(wo_sb[:], wo3)
    qT = [persist.tile([128, S], BF16, tag=f"qT{p}", name=f"qT{p}") for p in range(2)]
    kT = persist.tile([128, S], BF16, tag="kT")
    vstage = persist.tile([128, S], BF16, tag="vstage")   # rows 64:128 hold v
    vones = persist.tile([128, KO, 65], BF16, tag="vones")
    nc.gpsimd.memset(vones[:, :, 64:65], 1.0)

    cc_in = [dram.tile([HQ * D, 512], BF16,
                       name=f"cc_in{i}", tag=f"cci{i}") for i in range(ST)]
    cc_out = [dram.tile([H * D, 512], BF16, addr_space="Shared",
                        name=f"cc_out{i}", tag=f"cco{i}") for i in range(ST)]


    hidd_pool = es.enter_context(tc.tile_pool(name="hidd", bufs=2))
    mm_ps = es.enter_context(tc.tile_pool(name="mmps", bufs=2, space="PSUM"))
    pa_ps = es.enter_context(tc.tile_pool(name="paps", bufs=2, space="PSUM"))
    stage_pool = es.enter_context(tc.tile_pool(name="stage", bufs=5))
    et_pool = es.enter_context(tc.tile_pool(name="et", bufs=12))
    fin_pool = es.enter_context(tc.tile_pool(name="fin", bufs=2))
    norm_pool = es.enter_context(tc.tile_pool(name="norm", bufs=2))
    af_pool = es.enter_context(tc.tile_pool(name="af", bufs=2))

    ht = [None, None]  # double-buffered hidden tiles

    def load_hidden(st, nchunks=1, spread=False):
        t = hidd_pool.tile([128, KO, 512], BF16, tag="ht", name=f"ht{st}")
        step = KO // nchunks
        for i, k0 in enumerate(range(0, KO, step)):
            eng = nc.gpsimd if (spread and i % 2 == 1) else nc.sync
            eng.dma_start(t[:, k0:k0 + step, :],
                          hT3[:, k0:k0 + step, st * 512:(st + 1) * 512])
        ht[st % 2] = t

    def rope(ps_ap, dst, n_half, st):
        """ps_ap: psum [64*n_half, 512] raw q or k. dst[:64*n_half, st*512:...].

        DVE: shuffle + 2 muls; Pool: final add (SBUF-only engine).
        """
        sl = slice(st * 512, (st + 1) * 512)
        npart = 64 * n_half
        sw = stage_pool.tile([128, 1024], F32, tag="stage",
                             name=f"sw{st}_{n_half}_{ps_ap.offset}")
        nc.vector.stream_shuffle(sw[0:npart, 0:512], ps_ap, SWAP_MASK)
        nc.vector.tensor_mul(sw[0:npart, 0:512], sw[0:npart, 0:512],
                             sinI[0:npart, sl])
        nc.vector.tensor_mul(sw[0:npart, 512:1024], ps_ap, cosI[0:npart, sl])
        nc.vector.tensor_add(dst[0:npart, sl], sw[0:npart, 0:512],
                             sw[0:npart, 512:1024])

    def qkv_block(st, fillers=()):
        """QKV projection + rope + v transpose for sequence tile st.

        `fillers` are callbacks emitting PE work (e.g. lagged av groups)
        between the chains so the PE has slack while ACT/DVE catch up.
        """
        fillers = list(fillers)
        if st + 1 < ST:
            load_hidden(st + 1)
        h = ht[st % 2]
        psA = mm_ps.tile([128, 1024], F32, tag="mm", name=f"qkvA{st}")
        psB = mm_ps.tile([128, 1024], F32, tag="mm", name=f"qkvB{st}")
        # fused k|v chain: weight cols 256:384 -> psum rows 0:64 k, 64:128 v
        for ko in range(KO):
            nc.tensor.matmul(psA[:, 0:512], wq_sb[:, ko, 256:384], h[:, ko, :],
                             start=(ko == 0), stop=(ko == KO - 1))
        rope(psA[0:64, 0:512], kT, 1, st)
        nc.vector.tensor_copy(vstage[64:128, st * 512:(st + 1) * 512],
                              psA[64:128, 0:512])
        if fillers:
            fillers.pop(0)()
        for ko in range(KO):
            nc.tensor.matmul(psA[:, 512:1024], wq_sb[:, ko, 0:128], h[:, ko, :],
                             start=(ko == 0), stop=(ko == KO - 1))
        rope(psA[:, 512:1024], qT[0], 2, st)
        if fillers:
            fillers.pop(0)()
        for ko in range(KO):
            nc.tensor.matmul(psB[:, 0:512], wq_sb[:, ko, 128:256], h[:, ko, :],
                             start=(ko == 0), stop=(ko == KO - 1))
        rope(psB[:, 0:512], qT[1], 2, st)
        if fillers:
            fillers.pop(0)()
        # duplicate roped k to partitions 64:128 (for head x=1 matmuls)
        nc.sync.dma_start(kT[64:128, st * 512:(st + 1) * 512],
                          kT[0:64, st * 512:(st + 1) * 512])
        # transpose v chunks into vones (pvt shares the mm psum tag)
        for ki in range(4 * st, 4 * st + 4):
            pvt = mm_ps.tile([128, 1024], F32, tag="mm", name=f"pvt{ki}")
            pvtb = pvt.bitcast(BF16)
            nc.tensor.transpose(pvtb[:, 0:64],
                                vstage[64:128, ki * 128:(ki + 1) * 128],
                                identhi[64:128, :], tile_position=(64, 0))
            nc.vector.tensor_copy(vones[:, ki, 0:64], pvtb[:, 0:64])
        for f in fillers:
            f()

    # attention state
    et_tiles = {}    # (qt, ki) -> ([et_hp0, et_hp1], lo)
    pa_tiles = {}    # (qt, hp) -> pa tile [65,1024] (x halves)

    def scores(qt, ki, direct0=False):
        """Scores MMs for (qt, ki): 4 matmuls -> 2 psum pair tiles.

        hp0 pair: DVE drain (mask fused) -> stage f32 -> ACT exp -> et bf16.
        hp1 pair: (DVE in-place mask if diag) -> ACT exp direct -> et bf16.
        With direct0 (stream tail), hp0 also takes the direct-ACT path so the
        DVE queue is clear for the next block's rope ops.
        """
        m = ki - 4 * qt
        lo = max(0, m) * 128
        n = 512 - lo
        ps = [mm_ps.tile([128, 1024], F32, tag="mm", name=f"ps{qt}_{ki}_{hp}")
              for hp in range(2)]
        for hp in range(2):
            for x in range(2):
                nc.tensor.matmul(
                    ps[hp][:, x * 512 + lo:(x + 1) * 512],
                    kT[x * 64:(x + 1) * 64, ki * 128:(ki + 1) * 128],
                    qT[hp][x * 64:(x + 1) * 64, qt * 512 + lo:(qt + 1) * 512],
                    start=True, stop=True)
        et_t = [et_pool.tile([128, 1024], BF16, tag="et", name=f"et{qt}_{ki}_{hp}")
                for hp in range(2)]
        mask3 = mask_sb[:, None, :].to_broadcast((128, 2, 128))

        # hp0: DVE drain to stage (mask fused), ACT exp from stage
        ps3 = ps[0].rearrange("p (two k) -> p two k", two=2)
        e3 = et_t[0].rearrange("p (two k) -> p two k", two=2)
        if direct0:
            if m >= 0:
                nc.vector.tensor_add(ps3[:, :, lo:lo + 128],
                                     ps3[:, :, lo:lo + 128], mask3)
            nc.scalar.activation(e3[:, :, 0:n], ps3[:, :, lo:512],
                                 AF.Exp, scale=0.125)
        else:
            sg = stage_pool.tile([128, 1024], F32, tag="stage", name=f"sg{qt}_{ki}")
            sg3 = sg.rearrange("p (two k) -> p two k", two=2)
            if m >= 0:
                nc.vector.tensor_add(sg3[:, :, 0:128], ps3[:, :, lo:lo + 128], mask3)
                if n > 128:
                    nc.vector.tensor_copy(sg3[:, :, 128:n], ps3[:, :, lo + 128:512])
            else:
                nc.vector.tensor_copy(sg3[:, :, 0:n], ps3[:, :, lo:512])
            nc.scalar.activation(e3[:, :, 0:n], sg3[:, :, 0:n], AF.Exp, scale=0.125)

        # hp1: in-place mask (diag only) then ACT exp straight from psum
        ps3b = ps[1].rearrange("p (two k) -> p two k", two=2)
        if m >= 0:
            nc.vector.tensor_add(ps3b[:, :, lo:lo + 128],
                                 ps3b[:, :, lo:lo + 128], mask3)
        e3b = et_t[1].rearrange("p (two k) -> p two k", two=2)
        nc.scalar.activation(e3b[:, :, 0:n], ps3b[:, :, lo:512], AF.Exp, scale=0.125)

        et_tiles[(qt, ki)] = (et_t, lo)

    def av(qt, ki, nki):
        et_t, lo = et_tiles.pop((qt, ki))
        n = 512 - lo
        for hp in range(2):
            if ki == 0:
                pa_tiles[(qt, hp)] = pa_ps.tile(
                    [65, 1024], F32, tag="pa", name=f"pa{qt}_{hp}")
            pa = pa_tiles[(qt, hp)]
            e3 = et_t[hp].rearrange("p (two k) -> p two k", two=2)
            for x in range(2):
                nc.tensor.matmul(
                    pa[:, x * 512 + lo:(x + 1) * 512],
                    vones[:, ki, :], e3[:, x, 0:n],
                    start=(ki == 0), stop=(ki == nki - 1))

    def norm_and_gather(qt):
        """Normalize the 4 heads of qt, DMA bf16 rows to cc_in, AllGather.

        """
        fin = fin_pool.tile([64, 4, 512], BF16, tag="fin", name=f"fin{qt}")
        for hp in range(2):
            pa = pa_tiles[(qt, hp)]
            zrow = norm_pool.tile([1, 1024], F32, tag="zrow", name=f"z{qt}_{hp}", bufs=2)
            nc.scalar.activation(zrow[:], pa[64:65, :], AF.Copy)
            rz = norm_pool.tile([1, 1024], F32, tag="rz", name=f"rz{qt}_{hp}", bufs=2)
            nc.vector.reciprocal_approx_fast(rz[:], zrow[:])
            for x in range(2):
                h = 2 * hp + x
                rbc = norm_pool.tile([64, 512], F32, tag="rbc",
                                     name=f"rbc{qt}_{h}", bufs=4)
                nc.gpsimd.partition_broadcast(rbc[:], rz[0:1, x * 512:(x + 1) * 512])
                nc.vector.tensor_mul(fin[:, h, :],
                                     pa[0:64, x * 512:(x + 1) * 512], rbc[:])
            pa_tiles.pop((qt, hp))
        # one DMA: sbuf [d:64, h:4, s:512] -> dram rows h*64+d of the chunk
        cc_v = cc_in[qt][:].rearrange("(h d) s -> d h s", h=HQ)
        nc.sync.dma_start(cc_v, fin[:])
        nc.gpsimd.collective_compute(
            "AllGather", mybir.AluOpType.bypass,
            ins=[cc_in[qt][:].opt()],
            outs=[cc_out[qt][:].opt()],
            replica_groups=[list(range(NCORES))],
        )

    af_tiles = {}

    def prefetch_af(qt):
        af = af_pool.tile([128, KO, 512], BF16, tag="af", name=f"af{qt}")
        co3 = cc_out[qt][:].rearrange("(ko p) s -> p ko s", p=128)
        for k0 in range(0, KO, 4):
            nc.sync.dma_start(af[:, k0:k0 + 4, :], co3[:, k0:k0 + 4, :])
        af_tiles[qt] = af

    def oproj(qt):
        af = af_tiles.pop(qt)
        po = mm_ps.tile([128, 1024], F32, tag="mm", name=f"po{qt}")
        order = [(ko, ko) for ko in range(KO)]
        for ft in range(2):
            for i, (slot, ko) in enumerate(order):
                nc.tensor.matmul(po[:, ft * 512:(ft + 1) * 512],
                                 wo_sb[:, ko, ft * 128:(ft + 1) * 128],
                                 af[:, slot, :], start=(i == 0), stop=(i == KO - 1))
        ot = fin_pool.tile([128, 1024], F32, tag="ot", name=f"ot{qt}", bufs=2)
        nc.scalar.activation(ot[:, 0:512], po[:, 0:512], AF.Copy)
        nc.vector.tensor_copy(ot[:, 512:1024], po[:, 512:1024])
        for ft in range(2):
            nc.sync.dma_start(
                outT[ft * 128:(ft + 1) * 128, qt * 512:(qt + 1) * 512],
                ot[:, ft * 512:(ft + 1) * 512])

    # ---- main schedule ----
    # Per qt: scores stream with av lagging LAG ki behind; the av tail is
    # interleaved into the next QKV block (ACT exp backlog drains under the
    # QKV matmuls); normalization + the AllGather chunk trigger follow. All
    # o_proj chunks run at the end: AG(0..2) complete long before, and AG(3)
    # hides under oproj(0..2).
    load_hidden(0, nchunks=4)
    qkv_block(0)
    for qt in range(ST):
        nki = 4 * qt + 4
        for ki in range(nki):
            scores(qt, ki, direct0=(ki >= nki - 2))
            if ki >= LAG:
                av(qt, ki - LAG, nki)
        tail = [(lambda k=ki: av(qt, k, nki))
                for ki in range(max(0, nki - LAG), nki)]
        if qt + 1 < ST:
            qkv_block(qt + 1, fillers=tail)
        else:
            prefetch_af(0)
            for f in tail:
                f()
        norm_and_gather(qt)
    for qt in range(ST):
        if qt + 1 < ST:
            prefetch_af(qt + 1)
        oproj(qt)
    es.close()


_CACHE = {}


def build_program():
    if "nc" in _CACHE:
        return _CACHE["nc"]
    nc = bacc.Bacc("TRN2", target_bir_lowering=False, debug=False,
                   enable_asserts=True, num_devices=NCORES)
    aps = {}
    aps["hiddenT"] = nc.dram_tensor("hiddenT", [HID, S], BF16, kind="ExternalInput").ap()
    aps["wqkvT"] = nc.dram_tensor("wqkvT", [HID, (HQ + 2) * D], BF16, kind="ExternalInput").ap()
    aps["woT"] = nc.dram_tensor("woT", [HID, HQ * D], BF16, kind="ExternalInput").ap()
    aps["trimask"] = nc.dram_tensor("trimask", [128, 128], F32, kind="ExternalInput").ap()
    aps["identhi"] = nc.dram_tensor("identhi", [128, 64], BF16, kind="ExternalInput").ap()
    aps["cos_t"] = nc.dram_tensor("cos_t", [128, S], F32, kind="ExternalInput").ap()
    aps["sin_t"] = nc.dram_tensor("sin_t", [128, S], F32, kind="ExternalInput").ap()
    aps["outT"] = nc.dram_tensor("outT", [HQ * D, S], F32, kind="ExternalOutput").ap()

    with tile.TileContext(nc) as tc:
        build_body(tc, aps)
    nc.compile()
    _CACHE["nc"] = nc
    return nc


def _interleave_rows(w):
    """Permute rope rows: per 64-row head block, row 2i <- i, row 2i+1 <- i+32."""
    blocks = []
    for h0 in range(0, w.shape[0], D):
        blk = w[h0:h0 + D]
        out = np.empty_like(blk)
        out[0::2] = blk[0:D // 2]
        out[1::2] = blk[D // 2:D]
        blocks.append(out)
    return np.concatenate(blocks, axis=0)


def make_in_maps(hidden_states, position_ids, powers, Wq, Wk, Wv, Wo):
    hidden = np.asarray(hidden_states, np.float32).reshape(S, HID)
    hiddenT = np.ascontiguousarray(hidden.T).astype(ml_dtypes.bfloat16)
    pos = np.asarray(position_ids, np.int32).reshape(S).astype(np.float32)
    pw = np.asarray(powers, np.float32).reshape(D // 2)
    Wq = np.asarray(Wq, np.float32)
    Wk = np.asarray(Wk, np.float32)
    Wv = np.asarray(Wv, np.float32)
    Wo = np.asarray(Wo, np.float32)

    kl = np.arange(128)[:, None]
    ql = np.arange(128)[None, :]
    trimask = np.where(kl <= ql, 0.0, NEG).astype(np.float32)
    identhi = np.zeros((128, 64), np.float32)
    identhi[64:128] = np.eye(64)
    identhi = identhi.astype(ml_dtypes.bfloat16)

    # rope tables, interleaved layout with sign baked into sin
    inv_freq = (1.0 / BASE ** (1.0 / (1.0 + np.exp(-pw)))).astype(np.float32)  # [32]
    freqs = inv_freq[:, None] * pos[None, :]                   # [32, S]
    c = np.cos(freqs).astype(np.float32)
    s = np.sin(freqs).astype(np.float32)
    cos_band = np.empty((64, S), np.float32)
    cos_band[0::2] = c
    cos_band[1::2] = c
    sin_band = np.empty((64, S), np.float32)
    sin_band[0::2] = -s
    sin_band[1::2] = s
    cos_t = np.ascontiguousarray(np.tile(cos_band, (2, 1)))
    sin_t = np.ascontiguousarray(np.tile(sin_band, (2, 1)))

    in_maps = []
    for cix in range(NCORES):
        wq_c = _interleave_rows(Wq[cix * HQ * D:(cix + 1) * HQ * D])   # [256, HID]
        wk_c = _interleave_rows(Wk[cix * D:(cix + 1) * D])             # [64, HID]
        wv_c = Wv[cix * D:(cix + 1) * D]                               # [64, HID]
        wqkv = np.concatenate([wq_c, wk_c, wv_c], axis=0)              # [384, HID]
        m = {
            "hiddenT": hiddenT,
            "wqkvT": np.ascontiguousarray(wqkv.T).astype(ml_dtypes.bfloat16),
            "woT": np.ascontiguousarray(
                Wo[cix * HQ * D:(cix + 1) * HQ * D].T).astype(ml_dtypes.bfloat16),
            "trimask": trimask,
            "identhi": identhi,
            "cos_t": cos_t,
            "sin_t": sin_t,
        }
        in_maps.append(m)
    return in_maps


def run_spmd(nc, in_maps, **kwargs):
    m = nc.m
    nc.m = get_hw_module(nc.m)
    try:
        return bass_utils.run_bass_kernel_spmd(
            nc, in_maps, core_ids=list(range(NCORES)), **kwargs)
    finally:
        nc.m = m


def kernel(hidden_states, position_ids, powers, Wq, Wk, Wv, Wo):
    nc = build_program()
    in_maps = make_in_maps(hidden_states, position_ids, powers, Wq, Wk, Wv, Wo)
    res = run_spmd(nc, in_maps)
    outT_full = np.concatenate([res.results[c]["outT"] for c in range(NCORES)], axis=0)
    return np.ascontiguousarray(outT_full.T).reshape(1, S, HID).astype(np.float32)


if __name__ == "__main__":
    rng = np.random.default_rng(0)
    inputs = {
        "hidden_states": rng.standard_normal((1, S, HID), dtype=np.float32),
        "position_ids": np.broadcast_to(np.arange(S, dtype=np.int32), (1, S)),
        "powers": rng.standard_normal(D // 2).astype(np.float32),
        "Wq": (rng.standard_normal((H * D, HID)) * 0.02).astype(np.float32),
        "Wk": (rng.standard_normal((KV * D, HID)) * 0.02).astype(np.float32),
        "Wv": (rng.standard_normal((KV * D, HID)) * 0.02).astype(np.float32),
        "Wo": (rng.standard_normal((HID, H * D)) * 0.02).astype(np.float32),
    }
    out = kernel(**inputs)
    print("out", out.shape, out.dtype, np.abs(out).max())
